# revision 11
# baseline (speedup 1.0000x reference)
"""Trainium2 Bass kernel for nn_BaseLUTLayer (soft-LUT layer).

Math: out[b,o] = sum_k lut[o,k] * prod_j (bit_j(k) ? x[b,m(o,j)] : 1-x[b,m(o,j)])

Strategy (per core, batch-sharded 8 ways, 128 batch rows each):
  * odds transform: with w = 1-x, r = x/(1-x):
        out[b,o] = (prod_j w_j) * H,   H = successive halving of lut with
        T_new[k'] = T_lo[k'] + r_j * T_hi[k']   (6 levels, 2 DVE ops/elem)
  * layout: nodes on SBUF partitions (o_p = o % 128), free dims (k', b).
    lut tiles live per-partition (no replication); r/w values are gathered
    per (node, wire) with dma_gather using compile-time indices derived
    from `mapping` (host-known at trace time).
  * gather source: G[row(i)] = [w[:,i] (128 f32) | r[:,i] (128 f32)] built
    on-device (clamp, 1-x, reciprocal, PE transposes) and bounced via HBM.
"""

import numpy as np

import concourse.bass as bass
import concourse.mybir as mybir
from concourse import bacc
from concourse import tile
from concourse.masks import make_identity
from concourse.bass_utils import run_bass_kernel_spmd

P = 128
IN = 1024
OUT = 2048
NB = 6
B_FULL = 1024
N_CORES = 8
OHI = OUT // P  # 16
F32 = mybir.dt.float32
I16 = mybir.dt.int16
# clamp x <= 1 - 2^-18 so r = x/(1-x) <= 2^18 and r^6 stays well inside fp32
CLAMP = float(1.0 - 2.0**-18)

# chunks of the o_hi loop assigned to gpsimd instead of DVE (load balance:
# gpsimd 2-input elementwise is ~2x slower than DVE, so give it ~1/3)
GPSIMD_CHUNKS = (5, 11)


def _mult():
    return mybir.AluOpType.mult


def _add():
    return mybir.AluOpType.add


def build_program():
    nc = bacc.Bacc("TRN2", target_bir_lowering=False, debug=False)

    xs = nc.dram_tensor("xs", [P, IN], F32, kind="ExternalInput").ap()
    gidx = nc.dram_tensor("gidx", [P, OUT * NB // 16], I16, kind="ExternalInput").ap()
    lut0 = nc.dram_tensor("lut0", [P, OHI, 32], F32, kind="ExternalInput").ap()
    lut1 = nc.dram_tensor("lut1", [P, OHI, 32], F32, kind="ExternalInput").ap()
    outs = nc.dram_tensor("outs", [P, OHI, P], F32, kind="ExternalOutput").ap()

    with tile.TileContext(nc) as tc:
        with (
            tc.tile_pool(name="consts", bufs=1) as consts,
            tc.tile_pool(name="main", bufs=1) as main,
            tc.tile_pool(name="zpool", bufs=3) as zpool,
            tc.tile_pool(name="tpool", bufs=2) as tpool,
            tc.tile_pool(name="spool", bufs=2) as spool,
            tc.tile_pool(name="psum", bufs=2, space="PSUM") as psum,
            tc.tile_pool(name="dram", bufs=1, space="DRAM") as dpool,
        ):
            ident = consts.tile([P, P], F32)
            make_identity(nc, ident)

            gidx_sb = consts.tile([P, OUT * NB // 16], I16)
            nc.sync.dma_start(gidx_sb, gidx)
            lut0_sb = consts.tile([P, OHI, 32], F32)
            nc.sync.dma_start(lut0_sb, lut0)
            lut1_sb = consts.tile([P, OHI, 32], F32)
            nc.sync.dma_start(lut1_sb, lut1)

            # x shard, clamped; w = 1-x; r = x * (1/w)
            xt = main.tile([P, IN], F32)
            nc.sync.dma_start(xt, xs)
            nc.vector.tensor_scalar_min(xt, xt, CLAMP)
            wt = main.tile([P, IN], F32)
            nc.vector.tensor_scalar(
                out=wt, in0=xt, scalar1=-1.0, scalar2=1.0, op0=_mult(), op1=_add()
            )
            rw = main.tile([P, IN], F32)
            nc.vector.reciprocal(rw, wt)
            rt = main.tile([P, IN], F32)
            nc.vector.tensor_mul(rt, xt, rw)

            # consolidate upfront deps (ident/wt/rt/consts) so the transpose
            # matmuls don't accumulate more sem waits than the ISA allows
            tc.strict_bb_all_engine_barrier()

            # transpose w/r into G rows: G[(i%128)*8 + i//128] = [w[:,i] | r[:,i]]
            gsb = main.tile([P, IN // P, 2 * P], F32)
            for ih in range(IN // P):
                pw = psum.tile([P, P], F32, tag="pt")
                nc.tensor.transpose(pw, wt[:, ih * P : (ih + 1) * P], ident)
                nc.scalar.copy(gsb[:, ih, 0:P], pw)
                pr = psum.tile([P, P], F32, tag="pt")
                nc.tensor.transpose(pr, rt[:, ih * P : (ih + 1) * P], ident)
                nc.scalar.copy(gsb[:, ih, P : 2 * P], pr)

            gd = dpool.tile([P * (IN // P), 2 * P], F32)
            gd_view = gd[:].rearrange("(p h) e -> p h e", h=IN // P)
            nc.sync.dma_start(gd_view, gsb)

            # main loop over node chunks (128 nodes each)
            idx_cols = NB * P // 16  # 48 idx columns per chunk
            for c in range(OHI):
                eng = nc.gpsimd if c in GPSIMD_CHUNKS else nc.vector
                z = zpool.tile([P, NB, 2 * P], F32, tag="z")
                nc.gpsimd.dma_gather(
                    out_ap=z,
                    in_ap=gd[:],
                    idxs_ap=gidx_sb[:, c * idx_cols : (c + 1) * idx_cols],
                    num_idxs=NB * P,
                    num_idxs_reg=NB * P,
                    elem_size=2 * P,
                )

                # level 1: T1[k'] = lut0 + r5 * lut1   (k' in [0,32))
                t = tpool.tile([P, 32, P], F32, tag="t1")
                eng.tensor_mul(
                    t,
                    z[:, 5, P : 2 * P][:, None, :].broadcast_to([P, 32, P]),
                    lut1_sb[:, c, :][:, :, None].broadcast_to([P, 32, P]),
                )
                eng.tensor_add(
                    t, t, lut0_sb[:, c, :][:, :, None].broadcast_to([P, 32, P])
                )

                # levels 2..6: T_new = T[:h] + r_j * T[h:2h]
                h = 16
                for j in (4, 3, 2, 1, 0):
                    tn = tpool.tile([P, h, P], F32, tag=f"t{h}")
                    eng.tensor_mul(
                        tn,
                        z[:, j, P : 2 * P][:, None, :].broadcast_to([P, h, P]),
                        t[:, h : 2 * h, :],
                    )
                    eng.tensor_add(tn, tn, t[:, 0:h, :])
                    t = tn
                    h //= 2

                # W = prod_j w_j ; out = W * T6
                w01 = spool.tile([P, P], F32, tag="w01")
                eng.tensor_mul(w01, z[:, 5, 0:P], z[:, 4, 0:P])
                w23 = spool.tile([P, P], F32, tag="w23")
                eng.tensor_mul(w23, z[:, 3, 0:P], z[:, 2, 0:P])
                w45 = spool.tile([P, P], F32, tag="w45")
                eng.tensor_mul(w45, z[:, 1, 0:P], z[:, 0, 0:P])
                eng.tensor_mul(w01, w01, w23)
                eng.tensor_mul(w01, w01, w45)
                ot = spool.tile([P, P], F32, tag="ot")
                eng.tensor_mul(ot, t[:, 0, :], w01)
                nc.sync.dma_start(outs[:, c, :], ot)

    # Bacc passes: event-sem generation (multi-wait lowering), auto library
    # loads for dma_gather, extended-InstISA byte packing, ...
    nc.compile()
    return nc


_CACHE: dict = {}


def _program():
    if "nc" not in _CACHE:
        _CACHE["nc"] = build_program()
    return _CACHE["nc"]


def make_inputs(x, lut_table, mapping):
    """Host-side input prep: shard x by batch, encode mapping as gather
    indices, split lut into node-on-partition lo/hi tiles."""
    x = np.ascontiguousarray(x, dtype=np.float32)
    lut_table = np.ascontiguousarray(lut_table, dtype=np.float32)
    mapping = np.asarray(mapping)

    # gather row of source column i: G row (i%128)*8 + i//128
    m3 = mapping.reshape(OHI, P, NB)  # [o_hi, o_p, j]
    rows = (m3 % P) * (IN // P) + (m3 // P)
    # t = (o_hi*NB + j)*128 + o_p  ->  order (o_hi, j, o_p)
    tvals = np.transpose(rows, (0, 2, 1)).reshape(-1)
    gidx16 = tvals.reshape(-1, 16).T.astype(np.int16)  # [16, OUT*NB/16]
    gidx_arr = np.ascontiguousarray(np.tile(gidx16, (P // 16, 1)))

    lut3 = lut_table.reshape(OHI, P, 64).transpose(1, 0, 2)  # [o_p, o_hi, 64]
    lut0_arr = np.ascontiguousarray(lut3[:, :, 0:32])
    lut1_arr = np.ascontiguousarray(lut3[:, :, 32:64])

    in_maps = []
    for core in range(N_CORES):
        in_maps.append(
            {
                "xs": np.ascontiguousarray(x[core * P : (core + 1) * P]),
                "gidx": gidx_arr,
                "lut0": lut0_arr,
                "lut1": lut1_arr,
            }
        )
    return in_maps


def assemble_output(results):
    """results: list of 8 dicts with 'outs' [128, 16, 128] -> full [1024, 2048]."""
    parts = []
    for core in range(N_CORES):
        arr = results[core]["outs"]  # [o_p, o_hi, b]
        parts.append(np.ascontiguousarray(arr.transpose(2, 1, 0).reshape(P, OUT)))
    return np.concatenate(parts, axis=0)


def kernel_with_results(x, lut_table, mapping, **kwargs):
    nc = _program()
    in_maps = make_inputs(x, lut_table, mapping)
    res = run_bass_kernel_spmd(nc, in_maps, core_ids=list(range(N_CORES)), **kwargs)
    return assemble_output(res.results), res


def kernel(x, lut_table, mapping):
    out, _ = kernel_with_results(x, lut_table, mapping)
    return out


if __name__ == "__main__":
    rng = np.random.default_rng(0)
    x = rng.random((B_FULL, IN), dtype=np.float32)
    lut = rng.standard_normal((OUT, 64), dtype=np.float32)
    mp = rng.integers(0, IN, (OUT, NB), dtype=np.int32)
    out = kernel(x, lut, mp)
    print(out.shape, out.dtype)


# revision 21
# speedup vs baseline: 1.5341x; 1.5341x over previous
"""Trainium2 Bass kernel for nn_BaseLUTLayer (soft-LUT layer).

Math: out[b,o] = sum_k lut[o,k] * prod_j (bit_j(k) ? x[b,m(o,j)] : 1-x[b,m(o,j)])

Strategy (per core, batch-sharded 8 ways, 128 batch rows each):
  * odds transform: with w = 1-x, r = x/(1-x):
        out[b,o] = (prod_j w_j) * H,   H = successive halving of lut with
        T_new[k'] = T_lo[k'] + r_j * T_hi[k']   (6 levels, 2 DVE ops/elem)
  * layout: nodes on SBUF partitions (o_p = o % 128), free dims (k', b).
    lut tiles live per-partition (no replication); r/w values are gathered
    per (node, wire) with dma_gather using compile-time indices derived
    from `mapping` (host-known at trace time).
  * gather source: G[row(i)] = [w[:,i] (128 f32) | r[:,i] (128 f32)] built
    on-device (clamp, 1-x, reciprocal, PE transposes) and bounced via HBM.
"""

import numpy as np

import concourse.bass as bass
import concourse.mybir as mybir
from concourse import bacc
from concourse import tile
from concourse.masks import make_identity
from concourse.bass_utils import run_bass_kernel_spmd

P = 128
IN = 1024
OUT = 2048
NB = 6
B_FULL = 1024
N_CORES = 8
OHI = OUT // P  # 16
F32 = mybir.dt.float32
I16 = mybir.dt.int16
# clamp x <= 1 - 2^-18 so r = x/(1-x) <= 2^18 and r^6 stays well inside fp32
CLAMP = float(1.0 - 2.0**-18)

# chunks of the o_hi loop assigned to gpsimd instead of DVE (load balance:
# gpsimd 2-input elementwise is ~2x slower than DVE, so give it ~1/3)
GPSIMD_CHUNKS = ()


def _mult():
    return mybir.AluOpType.mult


def _add():
    return mybir.AluOpType.add


def build_program():
    nc = bacc.Bacc("TRN2", target_bir_lowering=False, debug=False)

    xs = nc.dram_tensor("xs", [P, IN], F32, kind="ExternalInput").ap()
    gidx = nc.dram_tensor("gidx", [P, OUT * NB // 16], I16, kind="ExternalInput").ap()
    lut0 = nc.dram_tensor("lut0", [P, OHI, 32], F32, kind="ExternalInput").ap()
    lut1 = nc.dram_tensor("lut1", [P, OHI, 32], F32, kind="ExternalInput").ap()
    outs = nc.dram_tensor("outs", [P, OHI, P], F32, kind="ExternalOutput").ap()

    with tile.TileContext(nc) as tc:
        with (
            tc.tile_pool(name="consts", bufs=1) as consts,
            tc.tile_pool(name="main", bufs=1) as main,
            tc.tile_pool(name="zpool", bufs=3) as zpool,
            tc.tile_pool(name="tpool", bufs=2) as tpool,
            tc.tile_pool(name="spool", bufs=2) as spool,
            tc.tile_pool(name="dram", bufs=1, space="DRAM") as dpool,
        ):
            ident = consts.tile([P, P], F32)
            make_identity(nc, ident)

            gidx_sb = consts.tile([P, OUT * NB // 16], I16)
            nc.sync.dma_start(gidx_sb, gidx)
            lut0_sb = consts.tile([P, OHI, 32], F32)
            nc.sync.dma_start(lut0_sb, lut0)
            lut1_sb = consts.tile([P, OHI, 32], F32)
            nc.sync.dma_start(lut1_sb, lut1)

            # x shard, clamped; w = 1-x; r = x * (1/w)
            xt = main.tile([P, IN], F32)
            nc.sync.dma_start(xt, xs)
            nc.vector.tensor_scalar_min(xt, xt, CLAMP)
            wt = main.tile([P, IN], F32)
            nc.vector.tensor_scalar(
                out=wt, in0=xt, scalar1=-1.0, scalar2=1.0, op0=_mult(), op1=_add()
            )
            rw = main.tile([P, IN], F32)
            nc.vector.reciprocal(rw, wt)
            rt = main.tile([P, IN], F32)
            nc.vector.tensor_mul(rt, xt, rw)

            # consolidate upfront deps (ident/wt/rt/consts) so the transpose
            # matmuls don't accumulate more sem waits than the ISA allows
            tc.strict_bb_all_engine_barrier()

            # transpose w/r into G rows: G[(i%128)*8 + i//128] = [w[:,i] | r[:,i]]
            gsb = main.tile([P, IN // P, 2 * P], F32)
            with tc.tile_pool(name="psum_t", bufs=2, space="PSUM") as psum_t:
                for ih in range(IN // P):
                    pw = psum_t.tile([P, P], F32, tag="pt")
                    nc.tensor.transpose(pw, wt[:, ih * P : (ih + 1) * P], ident)
                    nc.scalar.copy(gsb[:, ih, 0:P], pw)
                    pr = psum_t.tile([P, P], F32, tag="pt")
                    nc.tensor.transpose(pr, rt[:, ih * P : (ih + 1) * P], ident)
                    nc.scalar.copy(gsb[:, ih, P : 2 * P], pr)

            gd = dpool.tile([P * (IN // P), 2 * P], F32)
            gd_view = gd[:].rearrange("(p h) e -> p h e", h=IN // P)
            nc.sync.dma_start(gd_view, gsb)

            # main loop over node chunks (128 nodes each)
            psum_cm = tc.tile_pool(name="psum", bufs=2, space="PSUM")
            psum = psum_cm.__enter__()
            idx_cols = NB * P // 16  # 48 idx columns per chunk
            for c in range(OHI):
                eng = nc.gpsimd if c in GPSIMD_CHUNKS else nc.vector
                z = zpool.tile([P, NB, 2 * P], F32, tag="z")
                nc.gpsimd.dma_gather(
                    out_ap=z,
                    in_ap=gd[:],
                    idxs_ap=gidx_sb[:, c * idx_cols : (c + 1) * idx_cols],
                    num_idxs=NB * P,
                    num_idxs_reg=NB * P,
                    elem_size=2 * P,
                )

                # level 1 (DVE): T1[k'] = lut0 + r5 * lut1   (k' in [0,32))
                t1 = tpool.tile([P, 32, P], F32, tag="t1")
                eng.tensor_mul(
                    t1,
                    z[:, 5, P : 2 * P][:, None, :].broadcast_to([P, 32, P]),
                    lut1_sb[:, c, :][:, :, None].broadcast_to([P, 32, P]),
                )
                eng.tensor_add(
                    t1, t1, lut0_sb[:, c, :][:, :, None].broadcast_to([P, 32, P])
                )
                t1f = t1[:].rearrange("p a b -> p (a b)")

                # level 2 (j=4): prod = r4*T1_hi (DVE); acc = T1_lo + prod via
                # identity-matmul accumulate on the otherwise-idle TensorE
                prod = tpool.tile([P, 16, P], F32, tag="pr16")
                eng.tensor_mul(
                    prod,
                    z[:, 4, P : 2 * P][:, None, :].broadcast_to([P, 16, P]),
                    t1[:, 16:32, :],
                )
                prodf = prod[:].rearrange("p a b -> p (a b)")
                acc = psum.tile([P, 16 * P], F32, tag="pacc")
                for s in range(4):
                    sl = slice(s * 512, (s + 1) * 512)
                    nc.tensor.matmul(
                        acc[:, sl], ident, t1f[:, sl], start=True, stop=False
                    )
                    nc.tensor.matmul(
                        acc[:, sl], ident, prodf[:, sl], start=False, stop=(s >= 2)
                    )

                # levels 3..4 (j=3,2; h=8,4):
                #   prod = r_j * acc[h:2h] (DVE, PSUM src); acc[0:h] += prod (PE)
                for j, h in ((3, 8), (2, 4)):
                    pn = tpool.tile([P, h, P], F32, tag=f"pr{h}")
                    eng.tensor_mul(
                        pn,
                        z[:, j, P : 2 * P][:, None, :].broadcast_to([P, h, P]),
                        acc[:, h * P : 2 * h * P].rearrange("p (a b) -> p a b", b=P),
                    )
                    pnf = pn[:].rearrange("p a b -> p (a b)")
                    w = min(512, h * P)
                    for s in range((h * P) // w):
                        sl = slice(s * w, (s + 1) * w)
                        nc.tensor.matmul(
                            acc[:, sl],
                            ident,
                            pnf[:, sl],
                            start=False,
                            stop=(h == 8 and s == 1) or (h == 4),
                        )

                # levels 5..6 (j=1,0; h=2,1) fully on DVE — the last PSUM zero
                # region must be closed (stop at h=4) before it can be read
                pn5 = tpool.tile([P, 2, P], F32, tag="pr2")
                eng.tensor_mul(
                    pn5,
                    z[:, 1, P : 2 * P][:, None, :].broadcast_to([P, 2, P]),
                    acc[:, 2 * P : 4 * P].rearrange("p (a b) -> p a b", b=P),
                )
                t5 = tpool.tile([P, 2, P], F32, tag="t5")
                eng.tensor_add(
                    t5, pn5, acc[:, 0 : 2 * P].rearrange("p (a b) -> p a b", b=P)
                )
                pn6 = tpool.tile([P, 1, P], F32, tag="pr1")
                eng.tensor_mul(
                    pn6,
                    z[:, 0, P : 2 * P][:, None, :].broadcast_to([P, 1, P]),
                    t5[:, 1:2, :],
                )
                t6 = tpool.tile([P, 1, P], F32, tag="t6")
                eng.tensor_add(t6, pn6, t5[:, 0:1, :])

                # W = prod_j w_j ; out = W * t6
                wp = spool.tile([P, 3, P], F32, tag="wp")
                eng.tensor_mul(wp, z[:, 1:6:2, 0:P], z[:, 0:5:2, 0:P])
                wq = spool.tile([P, P], F32, tag="wq")
                eng.tensor_mul(wq, wp[:, 0, :], wp[:, 1, :])
                eng.tensor_mul(wq, wq, wp[:, 2, :])
                ot = spool.tile([P, P], F32, tag="ot")
                eng.tensor_mul(ot, t6[:, 0, :], wq)
                nc.sync.dma_start(outs[:, c, :], ot)
            psum_cm.__exit__(None, None, None)

    # Bacc passes: event-sem generation (multi-wait lowering), auto library
    # loads for dma_gather, extended-InstISA byte packing, ...
    nc.compile()
    return nc


_CACHE: dict = {}


def _program():
    if "nc" not in _CACHE:
        _CACHE["nc"] = build_program()
    return _CACHE["nc"]


def make_inputs(x, lut_table, mapping):
    """Host-side input prep: shard x by batch, encode mapping as gather
    indices, split lut into node-on-partition lo/hi tiles."""
    x = np.ascontiguousarray(x, dtype=np.float32)
    lut_table = np.ascontiguousarray(lut_table, dtype=np.float32)
    mapping = np.asarray(mapping)

    # gather row of source column i: G row (i%128)*8 + i//128
    m3 = mapping.reshape(OHI, P, NB)  # [o_hi, o_p, j]
    rows = (m3 % P) * (IN // P) + (m3 // P)
    # t = (o_hi*NB + j)*128 + o_p  ->  order (o_hi, j, o_p)
    tvals = np.transpose(rows, (0, 2, 1)).reshape(-1)
    gidx16 = tvals.reshape(-1, 16).T.astype(np.int16)  # [16, OUT*NB/16]
    gidx_arr = np.ascontiguousarray(np.tile(gidx16, (P // 16, 1)))

    lut3 = lut_table.reshape(OHI, P, 64).transpose(1, 0, 2)  # [o_p, o_hi, 64]
    lut0_arr = np.ascontiguousarray(lut3[:, :, 0:32])
    lut1_arr = np.ascontiguousarray(lut3[:, :, 32:64])

    in_maps = []
    for core in range(N_CORES):
        in_maps.append(
            {
                "xs": np.ascontiguousarray(x[core * P : (core + 1) * P]),
                "gidx": gidx_arr,
                "lut0": lut0_arr,
                "lut1": lut1_arr,
            }
        )
    return in_maps


def assemble_output(results):
    """results: list of 8 dicts with 'outs' [128, 16, 128] -> full [1024, 2048]."""
    parts = []
    for core in range(N_CORES):
        arr = results[core]["outs"]  # [o_p, o_hi, b]
        parts.append(np.ascontiguousarray(arr.transpose(2, 1, 0).reshape(P, OUT)))
    return np.concatenate(parts, axis=0)


def kernel_with_results(x, lut_table, mapping, **kwargs):
    nc = _program()
    in_maps = make_inputs(x, lut_table, mapping)
    res = run_bass_kernel_spmd(nc, in_maps, core_ids=list(range(N_CORES)), **kwargs)
    return assemble_output(res.results), res


def kernel(x, lut_table, mapping):
    out, _ = kernel_with_results(x, lut_table, mapping)
    return out


if __name__ == "__main__":
    rng = np.random.default_rng(0)
    x = rng.random((B_FULL, IN), dtype=np.float32)
    lut = rng.standard_normal((OUT, 64), dtype=np.float32)
    mp = rng.integers(0, IN, (OUT, NB), dtype=np.int32)
    out = kernel(x, lut, mp)
    print(out.shape, out.dtype)
